# revision 1
# baseline (speedup 1.0000x reference)
"""Trainium2 Bass kernel for nn_ConditionedDense (hypernetwork-conditioned dense).

Reference computation:
    A = einsum('bnp,pq->bnq', P, Wk)         # hypernetwork: per-position weights
    W = relu(A).reshape(B, N, c_in, c_out)
    out = einsum('bni,bnio->bno', X, W)

Strategy: pure data parallel over 8 NeuronCores (shard batch dim). Per core
16384 positions, tiled 128 positions/tile, 4 tiles per DMA chunk:
  - PE matmul computes A-tile [128 pos, 1024] in PSUM (lhsT = P^T tile,
    rhs = Wk, both bf16; Wk host-permuted to q = o*32+i layout)
  - ACT applies relu (PSUM -> SBUF, bf16 out)
  - DVE (and GPSIMD for a fraction of tiles) multiplies by X broadcast
    over o; DVE grouped-reduces over i (innermost) and upcasts to fp32
Host side (free): P transposed per shard, Wk column-permuted, X/P/Wk cast
to bf16.
"""

import os
from contextlib import ExitStack

import numpy as np
import ml_dtypes

import concourse.bass as bass
import concourse.tile as tile
from concourse import bacc, mybir
from concourse.bass_utils import run_bass_kernel_spmd

C_IN = 32
C_OUT = 32
P_DIM = 64
Q = C_IN * C_OUT  # 1024
B, N = 32, 4096
N_CORES = 8
B_SH = B // N_CORES          # 4 batches per core
NPOS = B_SH * N              # 16384 positions per core
TILE_P = 128                 # positions per tile
N_TILES = NPOS // TILE_P     # 128
CHUNK = 8                    # tiles per DMA chunk
N_CHUNKS = N_TILES // CHUNK  # 16

F32 = mybir.dt.float32
BF16 = mybir.dt.bfloat16

_BUILD_CACHE = {}
LAST_RESULTS = None  # BassKernelResults of the most recent run (for profiling)


def _build_nc():
    nc = bacc.Bacc(
        "TRN2", target_bir_lowering=False, debug=False, num_devices=N_CORES
    )
    X_d = nc.declare_dram_parameter("X", [NPOS, C_IN], BF16, isOutput=False)
    PT_d = nc.declare_dram_parameter("PT", [P_DIM, NPOS], BF16, isOutput=False)
    Wk_d = nc.declare_dram_parameter("Wk", [P_DIM, Q], BF16, isOutput=False)
    out_d = nc.declare_dram_parameter("out", [NPOS, C_OUT], BF16, isOutput=True)

    relu = mybir.ActivationFunctionType.Relu
    mult = mybir.AluOpType.mult
    add = mybir.AluOpType.add

    with ExitStack() as ctx:
        tc = ctx.enter_context(tile.TileContext(nc))
        wkp = ctx.enter_context(tc.tile_pool(name="wk", bufs=1))
        xp = ctx.enter_context(tc.tile_pool(name="x", bufs=3))
        pp = ctx.enter_context(tc.tile_pool(name="pT", bufs=3))
        apool = ctx.enter_context(tc.tile_pool(name="apsum", bufs=2, space="PSUM"))
        wp = ctx.enter_context(tc.tile_pool(name="w", bufs=2))
        mp = ctx.enter_context(tc.tile_pool(name="m", bufs=2))
        t1p = ctx.enter_context(tc.tile_pool(name="t1", bufs=2))
        t2p = ctx.enter_context(tc.tile_pool(name="t2", bufs=2))
        t3p = ctx.enter_context(tc.tile_pool(name="t3", bufs=2))
        op = ctx.enter_context(tc.tile_pool(name="o", bufs=3))

        wk_t = wkp.tile([P_DIM, Q], BF16)
        nc.sync.dma_start(out=wk_t[:], in_=Wk_d[:])

        PAIR = 2   # tiles per PSUM tile / ACT relu op (PSUM tile = 4 banks)
        GRP = 8    # tiles fused per DVE op group (w tile spans 4 relu outputs)
        for ch in range(N_CHUNKS):
            # chunk loads: CHUNK * 128 positions per DMA
            x_c = xp.tile([TILE_P, CHUNK, C_IN], BF16)
            nc.sync.dma_start(
                out=x_c[:],
                in_=X_d[bass.ts(ch, TILE_P * CHUNK), :].rearrange(
                    "(a p) i -> p a i", p=TILE_P
                ),
            )
            pT_c = pp.tile([P_DIM, CHUNK * TILE_P], BF16)
            nc.sync.dma_start(
                out=pT_c[:], in_=PT_d[:, bass.ts(ch, TILE_P * CHUNK)]
            )
            o_c = op.tile([TILE_P, CHUNK, C_OUT], BF16)

            for g in range(CHUNK // GRP):
                # w tile spans GRP tiles; filled by GRP//PAIR relu ops
                w_t = wp.tile([TILE_P, GRP, Q], BF16)
                for h in range(GRP // PAIR):
                    a_t = apool.tile([TILE_P, PAIR, Q], F32)
                    for j in range(PAIR):
                        lhsT = pT_c[
                            :, bass.ts(g * GRP + h * PAIR + j, TILE_P)
                        ]
                        nc.tensor.matmul(
                            a_t[:, j, 0:512], lhsT=lhsT, rhs=wk_t[:, 0:512],
                            start=True, stop=True,
                        )
                        nc.tensor.matmul(
                            a_t[:, j, 512:1024], lhsT=lhsT,
                            rhs=wk_t[:, 512:1024], start=True, stop=True,
                        )
                    # relu: PSUM -> SBUF, cast to bf16 (ACT), 2 tiles/op
                    nc.scalar.activation(
                        w_t[:, bass.ts(h, PAIR), :], a_t[:], relu
                    )

                # m[p, j, o, i] = w[p, j, o, i] * x[p, j, i]   (DVE, 2x bf16)
                m_t = mp.tile([TILE_P, GRP, Q], BF16)
                w4 = w_t[:].rearrange("p j (o i) -> p j o i", o=C_OUT)
                m4 = m_t[:].rearrange("p j (o i) -> p j o i", o=C_OUT)
                x4 = x_c[:, bass.ts(g, GRP), :].unsqueeze(2).broadcast_to(
                    [TILE_P, GRP, C_OUT, C_IN]
                )
                nc.vector.tensor_tensor(out=m4, in0=w4, in1=x4, op=mult)

                # Reduce over i (innermost, 32 wide). TensorReduce has no
                # 2x DVE mode, so halve twice with 2x TENSOR_TENSOR adds,
                # then one short reduce.
                t1 = t1p.tile([TILE_P, GRP, C_OUT, 16], BF16)
                nc.vector.tensor_tensor(
                    out=t1[:], in0=m4[:, :, :, 0:16], in1=m4[:, :, :, 16:32],
                    op=add,
                )
                t2 = t2p.tile([TILE_P, GRP, C_OUT, 8], BF16)
                nc.vector.tensor_tensor(
                    out=t2[:], in0=t1[:, :, :, 0:8], in1=t1[:, :, :, 8:16],
                    op=add,
                )
                t3 = t3p.tile([TILE_P, GRP, C_OUT, 4], BF16)
                nc.vector.tensor_tensor(
                    out=t3[:], in0=t2[:, :, :, 0:4], in1=t2[:, :, :, 4:8],
                    op=add,
                )
                with nc.allow_low_precision("bf16 reduce, fp32 internal accum"):
                    nc.vector.tensor_reduce(
                        out=o_c[:, bass.ts(g, GRP), :], in_=t3[:],
                        axis=mybir.AxisListType.X, op=add,
                    )

            nc.sync.dma_start(
                out=out_d[bass.ts(ch, TILE_P * CHUNK), :].rearrange(
                    "(a p) i -> p a i", p=TILE_P
                ),
                in_=o_c[:],
            )

    nc.finalize()
    return nc


def _get_nc():
    key = "v2"
    if key not in _BUILD_CACHE:
        _BUILD_CACHE[key] = _build_nc()
    return _BUILD_CACHE[key]


def kernel(X, P, Wk):
    global LAST_RESULTS
    X = np.asarray(X, dtype=np.float32)
    P = np.asarray(P, dtype=np.float32)
    Wk = np.asarray(Wk, dtype=np.float32)
    bf16 = ml_dtypes.bfloat16

    # Host-side prep (free): shard, transpose P, permute Wk columns so the
    # device-side layout is q = o*32 + i; cast matmul operands to bf16.
    WkP = np.ascontiguousarray(
        Wk.reshape(P_DIM, C_IN, C_OUT).transpose(0, 2, 1).reshape(P_DIM, Q)
    ).astype(bf16)
    in_maps = []
    for c in range(N_CORES):
        Xc = np.ascontiguousarray(
            X[c * B_SH:(c + 1) * B_SH].reshape(NPOS, C_IN)
        ).astype(bf16)
        PTc = np.ascontiguousarray(
            P[c * B_SH:(c + 1) * B_SH].reshape(NPOS, P_DIM).T
        ).astype(bf16)
        in_maps.append({"X": Xc, "PT": PTc, "Wk": WkP})

    nc = _get_nc()
    trace = os.environ.get("BASS_PROFILE", "0") == "1"
    kw = {}
    if os.environ.get("BASS_TMPDIR"):
        kw["tmpdir"] = os.environ["BASS_TMPDIR"]
    res = run_bass_kernel_spmd(
        nc, in_maps, list(range(N_CORES)), trace=trace, **kw
    )
    LAST_RESULTS = res

    out = np.empty((B, N, C_OUT), dtype=np.float32)
    for c in range(N_CORES):
        out[c * B_SH:(c + 1) * B_SH] = (
            np.asarray(res.results[c]["out"])
            .astype(np.float32)
            .reshape(B_SH, N, C_OUT)
        )
    return out



# revision 3
# speedup vs baseline: 1.1668x; 1.1668x over previous
"""Trainium2 Bass kernel for nn_ConditionedDense (hypernetwork-conditioned dense).

Reference computation:
    A = einsum('bnp,pq->bnq', P, Wk)         # hypernetwork: per-position weights
    W = relu(A).reshape(B, N, c_in, c_out)
    out = einsum('bni,bnio->bno', X, W)

Strategy (v2): pure data parallel over 8 NeuronCores (shard batch dim),
A^T-oriented dataflow so both einsums run on the PE with static weights:

  - A^T layout: [q' partitions, pos free] with q' = o*32 + i.  PE computes
    A^T chunks (128 q' x T pos) with lhsT = Wk' chunk (static), rhs = P^T.
    K=64 -> two chunks run concurrently via row tiling (rows 0-63 / 64-127),
    with P^T duplicated on partitions 64-127.
  - m = relu(A) * X: split between two paths to balance ACT and DVE:
      ACT path: scalar.activation(Relu) PSUM->SBUF bf16, then DVE
                tensor_tensor mult (2x bf16) by X replicated 4x on partitions
                (X_rep[p,t] = X[t, p%32], chunk-independent).
      DVE path: fused scalar_tensor_tensor (max 0, mult) straight from PSUM.
  - reduce over i on the PE: 8 accumulating matmuls per tile with static
    0/1 selection weights S_c[p, o] = (o == 4c + p//32), output col-tiled
    into out^T PSUM [32j:32j+32, :] for tile j of each supergroup of 4.
  - out stays in packed transposed layout; host unpacks (free).

Host side (free): P^T duplicated x2, X^T replicated x4, Wk column-permuted
to q' = o*32+i and packed into row-tiled pairs, S selection matrices, all
cast to bf16.
"""

import os
from contextlib import ExitStack

import numpy as np
import ml_dtypes

import concourse.bass as bass
import concourse.tile as tile
from concourse import bacc, mybir
from concourse.bass_utils import run_bass_kernel_spmd

C_IN = 32
C_OUT = 32
P_DIM = 64
Q = C_IN * C_OUT             # 1024
B, N = 32, 4096
N_CORES = 8
B_SH = B // N_CORES          # 4 batches per core
NPOS = B_SH * N              # 16384 positions per core
T = 512                      # positions per tile (matmul N)
TILES = NPOS // T            # 32
SG_TILES = 4                 # tiles per supergroup (col-tiled out^T group)
N_SG = TILES // SG_TILES     # 8
T_SG = T * SG_TILES          # 2048 positions per supergroup
PAIRS = 4                    # chunk pairs per tile (8 q'-chunks of 128)
ACT_PAIRS = 3                # pairs on ACT-relu + DVE-mult path; rest fused DVE

F32 = mybir.dt.float32
BF16 = mybir.dt.bfloat16

_BUILD_CACHE = {}
LAST_RESULTS = None  # BassKernelResults of the most recent run (for profiling)


def _build_nc():
    nc = bacc.Bacc(
        "TRN2", target_bir_lowering=False, debug=False, num_devices=N_CORES
    )
    XR_d = nc.declare_dram_parameter("XR", [N_SG * 128, T_SG], BF16, isOutput=False)
    P2_d = nc.declare_dram_parameter("P2", [N_SG * 128, T_SG], BF16, isOutput=False)
    WK_d = nc.declare_dram_parameter("WK", [128, PAIRS * 128], BF16, isOutput=False)
    S_d = nc.declare_dram_parameter("S", [128, 8 * C_OUT], BF16, isOutput=False)
    out_d = nc.declare_dram_parameter("out", [N_SG * 128, T], BF16, isOutput=True)

    relu = mybir.ActivationFunctionType.Relu
    copyf = mybir.ActivationFunctionType.Copy
    mult = mybir.AluOpType.mult
    amax = mybir.AluOpType.max

    with ExitStack() as ctx:
        tc = ctx.enter_context(tile.TileContext(nc))
        wkp = ctx.enter_context(tc.tile_pool(name="wk", bufs=1))
        ssp = ctx.enter_context(tc.tile_pool(name="sel", bufs=1))
        xrp = ctx.enter_context(tc.tile_pool(name="xr", bufs=2))
        p2p = ctx.enter_context(tc.tile_pool(name="p2", bufs=2))
        apool = ctx.enter_context(tc.tile_pool(name="apsum", bufs=3, space="PSUM"))
        wpool = ctx.enter_context(tc.tile_pool(name="w", bufs=3))
        mpool = ctx.enter_context(tc.tile_pool(name="m", bufs=4))
        opool = ctx.enter_context(tc.tile_pool(name="opsum", bufs=2, space="PSUM"))
        obp = ctx.enter_context(tc.tile_pool(name="osb", bufs=2))

        wk_t = wkp.tile([128, PAIRS, 128], BF16)
        nc.sync.dma_start(
            out=wk_t[:], in_=WK_d[:].rearrange("p (a b) -> p a b", a=PAIRS)
        )
        s_t = ssp.tile([128, 8, C_OUT], BF16)
        nc.sync.dma_start(
            out=s_t[:], in_=S_d[:].rearrange("p (a b) -> p a b", a=8)
        )

        for sg in range(N_SG):
            xr = xrp.tile([128, T_SG], BF16)
            p2 = p2p.tile([128, T_SG], BF16)
            # split loads over partition quarters -> spread across DMA queues
            for qd in range(4):
                r0 = sg * 128 + 32 * qd
                nc.sync.dma_start(
                    out=xr[32 * qd:32 * (qd + 1), :], in_=XR_d[r0:r0 + 32, :]
                )
                nc.sync.dma_start(
                    out=p2[32 * qd:32 * (qd + 1), :], in_=P2_d[r0:r0 + 32, :]
                )

            ot = opool.tile([128, T], F32)
            for j in range(SG_TILES):
                js = bass.ts(j, T)
                for p in range(PAIRS):
                    psA = apool.tile([128, 2, T], F32)
                    nc.tensor.matmul(
                        psA[:, 0, :], lhsT=wk_t[0:64, p, :], rhs=p2[0:64, js],
                        start=True, stop=True,
                    )
                    nc.tensor.matmul(
                        psA[:, 1, :], lhsT=wk_t[64:128, p, :],
                        rhs=p2[64:128, js], start=True, stop=True,
                    )
                    m = mpool.tile([128, 2, T], BF16)
                    x_in = xr[:, js].unsqueeze(1).broadcast_to([128, 2, T])
                    if p < ACT_PAIRS:
                        w = wpool.tile([128, 2, T], BF16)
                        nc.scalar.activation(w[:], psA[:], relu)
                        nc.vector.tensor_tensor(
                            out=m[:], in0=w[:], in1=x_in, op=mult
                        )
                    else:
                        nc.vector.scalar_tensor_tensor(
                            out=m[:], in0=psA[:], scalar=0.0, in1=x_in,
                            op0=amax, op1=mult,
                        )
                    for c2 in range(2):
                        chunk = 2 * p + c2
                        nc.tensor.matmul(
                            ot[32 * j:32 * (j + 1), :],
                            lhsT=s_t[:, chunk, :], rhs=m[:, c2, :],
                            start=(chunk == 0), stop=(chunk == 7),
                            tile_position=(0, 32 * j),
                        )

            osb = obp.tile([128, T], BF16)
            nc.scalar.activation(osb[:], ot[:], copyf)
            nc.sync.dma_start(
                out=out_d[sg * 128:(sg + 1) * 128, :], in_=osb[:]
            )

    nc.finalize()
    return nc


def _get_nc():
    key = "v2"
    if key not in _BUILD_CACHE:
        _BUILD_CACHE[key] = _build_nc()
    return _BUILD_CACHE[key]


def _host_prep(X, P, Wk):
    """Build per-core input arrays (host-side prep is free)."""
    bf16 = ml_dtypes.bfloat16
    # Wk' with q' = o*32 + i
    WkP = np.ascontiguousarray(
        Wk.reshape(P_DIM, C_IN, C_OUT).transpose(0, 2, 1).reshape(P_DIM, Q)
    )
    # packed row-tiled pairs: [128, PAIRS, 128] -> [128, PAIRS*128]
    wk2 = np.zeros((128, PAIRS, 128), dtype=np.float32)
    for p in range(PAIRS):
        wk2[0:64, p, :] = WkP[:, 256 * p:256 * p + 128]
        wk2[64:128, p, :] = WkP[:, 256 * p + 128:256 * p + 256]
    WK_h = np.ascontiguousarray(wk2.reshape(128, PAIRS * 128)).astype(bf16)

    # S selection: S[pr, c, o] = 1 if o == 4c + pr//32
    pr = np.arange(128)[:, None, None]
    cc = np.arange(8)[None, :, None]
    oo = np.arange(C_OUT)[None, None, :]
    S = (oo == 4 * cc + pr // 32).astype(np.float32)
    S_h = np.ascontiguousarray(S.reshape(128, 8 * C_OUT)).astype(bf16)

    in_maps = []
    for c in range(N_CORES):
        Xc = np.ascontiguousarray(
            X[c * B_SH:(c + 1) * B_SH].reshape(NPOS, C_IN)
        )
        Pc = np.ascontiguousarray(
            P[c * B_SH:(c + 1) * B_SH].reshape(NPOS, P_DIM)
        )
        # X_rep [128, NPOS]: row pr = X[:, pr % 32]; then [sg] blocks
        XRc = np.tile(Xc.T, (4, 1))                    # [128, NPOS]
        XR_h = np.ascontiguousarray(
            XRc.reshape(128, N_SG, T_SG).transpose(1, 0, 2).reshape(
                N_SG * 128, T_SG
            )
        ).astype(bf16)
        # P^T duplicated x2 on partitions
        P2c = np.tile(Pc.T, (2, 1))                    # [128, NPOS]
        P2_h = np.ascontiguousarray(
            P2c.reshape(128, N_SG, T_SG).transpose(1, 0, 2).reshape(
                N_SG * 128, T_SG
            )
        ).astype(bf16)
        in_maps.append({"XR": XR_h, "P2": P2_h, "WK": WK_h, "S": S_h})
    return in_maps


def kernel(X, P, Wk):
    global LAST_RESULTS
    X = np.asarray(X, dtype=np.float32)
    P = np.asarray(P, dtype=np.float32)
    Wk = np.asarray(Wk, dtype=np.float32)

    in_maps = _host_prep(X, P, Wk)

    nc = _get_nc()
    trace = os.environ.get("BASS_PROFILE", "0") == "1"
    kw = {}
    if os.environ.get("BASS_TMPDIR"):
        kw["tmpdir"] = os.environ["BASS_TMPDIR"]
    res = run_bass_kernel_spmd(
        nc, in_maps, list(range(N_CORES)), trace=trace, **kw
    )
    LAST_RESULTS = res

    out = np.empty((B, N, C_OUT), dtype=np.float32)
    for c in range(N_CORES):
        # packed out^T: [sg*128 + 32j + o, t] -> pos = sg*T_SG + j*T + t
        o_c = (
            np.asarray(res.results[c]["out"])
            .astype(np.float32)
            .reshape(N_SG, SG_TILES, C_OUT, T)
            .transpose(0, 1, 3, 2)
            .reshape(B_SH, N, C_OUT)
        )
        out[c * B_SH:(c + 1) * B_SH] = o_c
    return out
